# revision 7
# baseline (speedup 1.0000x reference)
"""CQAttention (context-query attention) Bass kernel for 8 NeuronCores.

Full inputs:  C [64,128,1000] f32, Q [64,128,100] f32, W [64000,1,384] f32
Full output:  [64, 512, 1000] f32

Sharding: pure data-parallel on the batch dim — 8 batches per core.

Per-batch math (D=128, Lc=1000, Lq=100):
  Ct = C.T [Lc,D], Qt = Q.T [Lq,D], w1/w2/w3 = W row blocks [Lc,D]
  U  = w1 + w3*Ct ; v = rowsum(w2*Ct)
  S  = U @ Q + v  (the v term drops out of the row softmax S1)
  S1 = softmax_cols(S)  ; S2 = softmax_rows(S)
  A  = S1 @ Qt ; Bm = S1 @ (S2^T @ Ct)
  out = concat([Ct, A, Ct*A, Ct*Bm], 1).T  -> [4D, Lc]

On-chip orientation: scores are built transposed (S0T [Lq, Lc]) so the big
matmuls have a wide (>=256) moving free dim; the S2 path (exp(S0+v)) is
obtained by PE-transposing exp(S0T) tiles and scaling by exp(v) per
partition; the S1 normalization is applied via a gpsimd partition-broadcast
of 1/rowsum.
"""

import numpy as np

B, D, LC, LQ = 64, 128, 1000, 100
NCORES = 8
NB = B // NCORES   # batches per core
NT = 8             # LC tiles
TL = LC // NT      # 125

# dtype knobs for performance/precision trade (set via _build args)
_cache = {}


def _build(mm_f32r=True, tu_bf16=False, tr_f32r=True):
    import concourse.bass as bass
    import concourse.tile as tile
    from concourse import bacc, mybir, masks
    from contextlib import ExitStack

    f32 = mybir.dt.float32
    f32r = mybir.dt.float32r
    bf16 = mybir.dt.bfloat16
    AF = mybir.ActivationFunctionType
    ALU = mybir.AluOpType
    AX = mybir.AxisListType

    mmdt = f32r if mm_f32r else f32   # dtype for big-matmul operand tiles

    def rt(ap):
        return ap

    nc = bacc.Bacc("TRN2", target_bir_lowering=False, debug=False,
                   num_devices=NCORES)
    C_d = nc.dram_tensor("C", [NB, D, LC], f32, kind="ExternalInput").ap()
    Q_d = nc.dram_tensor("Q", [NB, D, LQ], f32, kind="ExternalInput").ap()
    W_d = nc.dram_tensor("W", [NB, LC, 3 * D], f32, kind="ExternalInput").ap()
    O_d = nc.dram_tensor("OUT", [NB, 4 * D, LC], f32, kind="ExternalOutput").ap()

    with tile.TileContext(nc) as tc, ExitStack() as ctx:
        const_pool = ctx.enter_context(tc.tile_pool(name="const", bufs=1))
        ident = const_pool.tile([128, 128], f32)
        masks.make_identity(nc, ident[:])
        identr = ident[:]

        sb = ctx.enter_context(tc.tile_pool(name="sb", bufs=2))
        small = ctx.enter_context(tc.tile_pool(name="small", bufs=2))
        outp = ctx.enter_context(tc.tile_pool(name="outp", bufs=2))
        tp_ps = ctx.enter_context(tc.tile_pool(name="tp_ps", bufs=4, space="PSUM"))
        mm_ps = ctx.enter_context(tc.tile_pool(name="mm_ps", bufs=2, space="PSUM"))

        for b in range(NB):
            # ---- loads ----
            w_sb = sb.tile([TL, NT * 3 * D], f32, tag="w", name=f"w{b}")
            nc.sync.dma_start(
                w_sb[:].rearrange("p (t c) -> p t c", c=3 * D),
                W_d[b].rearrange("(t p) c -> p t c", p=TL))
            c_sb = sb.tile([D, LC], f32, tag="c", name=f"c{b}")
            nc.sync.dma_start(c_sb[:], C_d[b])
            q_sb = sb.tile([D, LQ], f32, tag="q", name=f"q{b}")
            nc.sync.dma_start(q_sb[:], Q_d[b])

            # ---- Ct tiles (with a ones column per tile for s2) ----
            ct_sb = sb.tile([TL, NT * (D + 1)], f32, tag="ct", name=f"ct{b}")
            ctv = ct_sb[:].rearrange("p (t c) -> p t c", c=D + 1)
            nc.vector.memset(ctv[:, :, D:D + 1], 1.0)
            for g in range(2):
                ctp = tp_ps.tile([TL, 4 * D], f32, tag="tp", name=f"ctp{b}_{g}")
                for k in range(4):
                    t = 4 * g + k
                    nc.tensor.transpose(
                        ctp[:, k * D:(k + 1) * D],
                        c_sb[:, t * TL:(t + 1) * TL], identr)
                nc.scalar.activation(
                    ctv[:, 4 * g:4 * g + 4, 0:D],
                    ctp[:].rearrange("p (k c) -> p k c", c=D), AF.Copy)

            wv = w_sb[:].rearrange("p (t c) -> p t c", c=3 * D)
            w1 = wv[:, :, 0:D]
            w2 = wv[:, :, D:2 * D]
            w3 = wv[:, :, 2 * D:3 * D]
            ctd = ctv[:, :, 0:D]

            # ---- U = w1 + w3*Ct (tiled), v = rowsum(w2*Ct) ----
            w3ct = sb.tile([TL, NT * D], f32, tag="w3ct", name=f"w3ct{b}")
            w3ctv = w3ct[:].rearrange("p (t c) -> p t c", c=D)
            nc.vector.tensor_tensor(out=w3ctv, in0=w3, in1=ctd, op=ALU.mult)
            u_all = sb.tile([TL, NT * D], f32, tag="u", name=f"u{b}")
            u_allv = u_all[:].rearrange("p (t c) -> p t c", c=D)
            nc.gpsimd.tensor_tensor(out=u_allv, in0=w3ctv, in1=w1, op=ALU.add)

            vtmp = sb.tile([TL, NT * D], f32, tag="vtmp", name=f"vtmp{b}")
            vtmpv = vtmp[:].rearrange("p (t c) -> p t c", c=D)
            nc.gpsimd.tensor_tensor(out=vtmpv, in0=w2, in1=ctd, op=ALU.mult)
            v_all = small.tile([TL, NT], f32, tag="v", name=f"v{b}")
            nc.vector.tensor_reduce(v_all[:], vtmpv, axis=AX.X, op=ALU.add)
            expv = small.tile([TL, NT], f32, tag="expv", name=f"expv{b}")
            nc.scalar.activation(expv[:], v_all[:], AF.Exp)
            env = small.tile([TL, NT], f32, tag="env", name=f"env{b}")
            nc.scalar.activation(env[:], v_all[:], AF.Exp, scale=-1.0)

            # ---- U^T via PE transposes ----
            ut_sb = sb.tile([D, LC], mmdt, tag="ut", name=f"ut{b}")
            for g in range(2):
                utp = tp_ps.tile([D, 4 * TL], f32, tag="tp", name=f"utp{b}_{g}")
                for k in range(4):
                    t = 4 * g + k
                    nc.tensor.transpose(
                        utp[:, k * TL:(k + 1) * TL],
                        u_all[:, t * D:(t + 1) * D],
                        identr[0:TL, 0:TL])
                nc.scalar.activation(ut_sb[:, g * 500:(g + 1) * 500],
                                     utp[:], AF.Copy)

            # ---- S0T = (U @ Q)^T = Q^T-contracted matmul, [Lq, Lc] ----
            q_r = small.tile([D, LQ], mmdt, tag="q_r", name=f"q_r{b}")
            nc.scalar.activation(q_r[:], q_sb[:], AF.Copy)
            s0t = mm_ps.tile([LQ, LC], f32, tag="mm", name=f"s0t{b}")
            nc.tensor.matmul(s0t[:, 0:512], q_r[:], ut_sb[:, 0:512],
                             start=True, stop=True)
            nc.tensor.matmul(s0t[:, 512:1000], q_r[:], ut_sb[:, 512:1000],
                             start=True, stop=True)

            # ---- E1T = exp(S0T) ----
            e1t_sb = sb.tile([LQ, LC], f32, tag="e1t", name=f"e1t{b}")
            nc.scalar.activation(e1t_sb[:], s0t[:], AF.Exp)

            # ---- E2 tiles = transpose(E1T) * exp(v) ----
            e2_dt = bf16 if tu_bf16 else f32
            e2_all = sb.tile([TL, NT * LQ], e2_dt, tag="e2", name=f"e2{b}")
            e2v = e2_all[:].rearrange("p (t c) -> p t c", c=LQ)
            for g in range(2):
                e1p = tp_ps.tile([TL, 4 * LQ], f32, tag="tp", name=f"e1p{b}_{g}")
                for k in range(4):
                    t = 4 * g + k
                    nc.tensor.transpose(
                        e1p[:, k * LQ:(k + 1) * LQ],
                        e1t_sb[:, t * TL:(t + 1) * TL],
                        identr[0:LQ, 0:LQ])
                scl = expv[:, 4 * g:4 * g + 4].unsqueeze(-1).to_broadcast(
                    (TL, 4, LQ))
                nc.vector.tensor_tensor(
                    out=e2v[:, 4 * g:4 * g + 4, :],
                    in0=e1p[:].rearrange("p (k c) -> p k c", c=LQ),
                    in1=scl, op=ALU.mult)

            # ---- s1 path: s1 = rowsum(E1) = rowsum(E2)*exp(-v) ----
            e2rs = small.tile([TL, NT], f32, tag="e2rs", name=f"e2rs{b}")
            nc.vector.tensor_reduce(e2rs[:], e2v, axis=AX.X, op=ALU.add)
            s1a = small.tile([TL, NT], f32, tag="s1a", name=f"s1a{b}")
            nc.vector.tensor_tensor(out=s1a[:], in0=e2rs[:], in1=env[:],
                                    op=ALU.mult)
            s1ra = small.tile([TL, NT], f32, tag="s1ra", name=f"s1ra{b}")
            nc.vector.reciprocal(s1ra[:], s1a[:])
            s1rp = tp_ps.tile([NT, TL], f32, tag="tp", name=f"s1rp{b}")
            nc.tensor.transpose(s1rp[:], s1ra[:], identr[0:TL, 0:TL])
            s1st = small.tile([NT, TL], f32, tag="s1st", name=f"s1st{b}")
            nc.scalar.activation(s1st[:], s1rp[:], AF.Copy)
            s1row = small.tile([1, LC], f32, tag="s1row", name=f"s1row{b}")
            nc.sync.dma_start(s1row[:], s1st[:])

            # broadcast 1/s1 over Lq partitions, then S1T = E1T * (1/s1)
            s1bc = sb.tile([LQ, LC], f32, tag="s1bc", name=f"s1bc{b}")
            nc.gpsimd.partition_broadcast(s1bc[:], s1row[:], channels=LQ)
            s1t = sb.tile([LQ, LC], mmdt, tag="s1t", name=f"s1t{b}")
            nc.vector.tensor_tensor(out=s1t[:], in0=e1t_sb[:], in1=s1bc[:],
                                    op=ALU.mult)

            # ---- Qt ----
            qtp = tp_ps.tile([LQ, D], f32, tag="tp", name=f"qtp{b}")
            nc.tensor.transpose(qtp[:], q_sb[:], identr)
            qt_sb = small.tile([LQ, D], mmdt, tag="qt", name=f"qt{b}")
            nc.scalar.activation(qt_sb[:], qtp[:], AF.Copy)

            # ---- Tu = E2^T @ [Ct | 1]  (accumulate over tiles) ----
            tu = tp_ps.tile([LQ, D + 1], f32, tag="tp", name=f"tu{b}")
            for t in range(NT):
                lhs = e2v[:, t, :]
                rhs = ctv[:, t, :]
                if tu_bf16:
                    rhs = rhs  # ct stays f32; mixed dtypes not allowed
                nc.tensor.matmul(tu[:], lhs, rhs,
                                 start=(t == 0), stop=(t == NT - 1))

            s2r = small.tile([LQ, 1], f32, tag="s2r", name=f"s2r{b}")
            nc.vector.reciprocal(s2r[:], tu[:, D:D + 1])
            that_sb = small.tile([LQ, D], mmdt, tag="that", name=f"that{b}")
            nc.vector.tensor_scalar_mul(that_sb[:], tu[:, 0:D], s2r[:])

            # ---- A^T = Qt^T-contracted; Bm^T = That-contracted ----
            at = mm_ps.tile([D, LC], f32, tag="mm", name=f"at{b}")
            nc.tensor.matmul(at[:, 0:512], qt_sb[:], s1t[:, 0:512],
                             start=True, stop=True)
            nc.tensor.matmul(at[:, 512:1000], qt_sb[:], s1t[:, 512:1000],
                             start=True, stop=True)
            bmt = mm_ps.tile([D, LC], f32, tag="mm", name=f"bmt{b}")
            nc.tensor.matmul(bmt[:, 0:512], that_sb[:], s1t[:, 0:512],
                             start=True, stop=True)
            nc.tensor.matmul(bmt[:, 512:1000], that_sb[:],
                             s1t[:, 512:1000], start=True, stop=True)

            # ---- outputs ----
            nc.sync.dma_start(O_d[b, 0:D], c_sb[:])
            oa = outp.tile([D, LC], f32, tag="oa", name=f"oa{b}")
            nc.scalar.activation(oa[:], at[:], AF.Copy)
            nc.sync.dma_start(O_d[b, D:2 * D], oa[:])
            oca = outp.tile([D, LC], f32, tag="oca", name=f"oca{b}")
            nc.vector.tensor_tensor(out=oca[:], in0=c_sb[:], in1=at[:],
                                    op=ALU.mult)
            nc.sync.dma_start(O_d[b, 2 * D:3 * D], oca[:])
            ocb = outp.tile([D, LC], f32, tag="ocb", name=f"ocb{b}")
            nc.vector.tensor_tensor(out=ocb[:], in0=c_sb[:], in1=bmt[:],
                                    op=ALU.mult)
            nc.sync.dma_start(O_d[b, 3 * D:4 * D], ocb[:])

    nc.compile()
    return nc


def _get_nc(**kw):
    key = tuple(sorted(kw.items()))
    if key not in _cache:
        _cache[key] = _build(**kw)
    return _cache[key]


def kernel(C, Q, W, **build_kw):
    from concourse import bass_utils

    C = np.ascontiguousarray(C, np.float32)
    Q = np.ascontiguousarray(Q, np.float32)
    Wr = np.ascontiguousarray(W, np.float32).reshape(NCORES, NB, LC, 3 * D)
    Cs = C.reshape(NCORES, NB, D, LC)
    Qs = Q.reshape(NCORES, NB, D, LQ)

    nc = _get_nc(**build_kw)
    in_maps = [{"C": Cs[i], "Q": Qs[i], "W": Wr[i]} for i in range(NCORES)]
    res = bass_utils.run_bass_kernel_spmd(nc, in_maps,
                                          core_ids=list(range(NCORES)))
    out = np.concatenate([res.results[i]["OUT"] for i in range(NCORES)], 0)
    return out.astype(np.float32)


# revision 10
# speedup vs baseline: 1.5332x; 1.5332x over previous
"""CQAttention (context-query attention) Bass kernel for 8 NeuronCores.

Full inputs:  C [64,128,1000] f32, Q [64,128,100] f32, W [64000,1,384] f32
Full output:  [64, 512, 1000] f32

Sharding: pure data-parallel on the batch dim - 8 batches per core.

Per-batch math (D=128, Lc=1000, Lq=100):
  Ct = C.T [Lc,D], Qt = Q.T [Lq,D], w1/w2/w3 = W row blocks [Lc,D]
  U  = w1 + w3*Ct ; v = rowsum(w2*Ct)
  S  = U @ Q + v  (the v term drops out of the row softmax S1)
  S1 = softmax_cols(S) ; S2 = softmax_rows(S)
  A  = S1 @ Qt ; Bm = S1 @ (S2^T @ Ct)
  out = concat([Ct, A, Ct*A, Ct*Bm], 1).T  -> [4D, Lc]

Layout notes:
 - Lc is tiled 8 x 125 with the INTERLEAVED mapping i = p*8 + t (p =
   partition, t = tile) so the W DMA reads 12KB contiguous per partition.
   All intermediate tensors with an Lc axis are kept in the permuted
   (t-major) order; the final output ops unpermute via strided APs.
 - Scores are built transposed (S0T [Lq, Lc]) so the big matmuls run with
   float32r operands at full PE rate (moving free dim >= 256).
 - S1 normalization: column sums of exp(S0T) via a ones-vector matmul,
   reciprocal, then a K=1 matmul broadcast across partitions.
 - S2 path: PE-transpose exp(S0T) tiles, scale by exp(v), cast bf16, and
   contract with bf16 Ct tiles (ones column appended for the s2 sums).
"""

import numpy as np

B, D, LC, LQ = 64, 128, 1000, 100
NCORES = 8
NB = B // NCORES   # batches per core
NT = 8             # LC tiles
TL = LC // NT      # 125

_cache = {}


def _build(tu_bf16=True, v_bf16=True):
    import concourse.bass as bass
    import concourse.tile as tile
    from concourse import bacc, mybir, masks
    from contextlib import ExitStack

    f32 = mybir.dt.float32
    f32r = mybir.dt.float32r
    bf16 = mybir.dt.bfloat16
    AF = mybir.ActivationFunctionType
    ALU = mybir.AluOpType
    AX = mybir.AxisListType

    ct_dt = bf16 if tu_bf16 else f32
    e2_dt = bf16 if tu_bf16 else f32

    nc = bacc.Bacc("TRN2", target_bir_lowering=False, debug=False,
                   num_devices=NCORES)
    C_d = nc.dram_tensor("C", [NB, D, LC], f32, kind="ExternalInput").ap()
    Q_d = nc.dram_tensor("Q", [NB, D, LQ], f32, kind="ExternalInput").ap()
    W_d = nc.dram_tensor("W", [NB, LC, 3 * D], f32, kind="ExternalInput").ap()
    O_d = nc.dram_tensor("OUT", [NB, 4 * D, LC], f32, kind="ExternalOutput").ap()

    with tile.TileContext(nc) as tc, ExitStack() as ctx:
        const_pool = ctx.enter_context(tc.tile_pool(name="const", bufs=1))
        ident = const_pool.tile([128, 128], f32)
        masks.make_identity(nc, ident[:])
        identr = const_pool.tile([128, 128], f32r)
        nc.scalar.activation(identr[:], ident[:], AF.Copy)
        # f32r ones for the s1 column-sum / broadcast matmuls
        ones_f = const_pool.tile([128, 1], f32)
        nc.vector.memset(ones_f[:], 1.0)
        ones_col = const_pool.tile([128, 1], f32r)
        nc.scalar.activation(ones_col[:], ones_f[:], AF.Copy)
        ones_rf = const_pool.tile([1, 128], f32)
        nc.vector.memset(ones_rf[:], 1.0)
        ones_row = const_pool.tile([1, 128], f32r)
        nc.scalar.activation(ones_row[:], ones_rf[:], AF.Copy)

        sb = ctx.enter_context(tc.tile_pool(name="sb", bufs=2))
        small = ctx.enter_context(tc.tile_pool(name="small", bufs=3))
        outp = ctx.enter_context(tc.tile_pool(name="outp", bufs=2))
        tp_ps = ctx.enter_context(tc.tile_pool(name="tp_ps", bufs=4, space="PSUM"))
        mm_ps = ctx.enter_context(tc.tile_pool(name="mm_ps", bufs=2, space="PSUM"))

        for b in range(NB):
            # ---- loads ----
            # w_sb[p, t, c] = W[b, p*8+t, c]  (12KB contiguous per partition)
            w_sb = sb.tile([TL, NT * 3 * D], f32, tag="w", name=f"w{b}")
            nc.sync.dma_start(
                w_sb[:].rearrange("p (t c) -> p t c", c=3 * D),
                W_d[b].rearrange("(p t) c -> p t c", t=NT))
            c_sb = sb.tile([D, LC], f32, tag="c", name=f"c{b}")
            nc.sync.dma_start(c_sb[:], C_d[b])
            q_sb = sb.tile([D, LQ], f32, tag="q", name=f"q{b}")
            nc.sync.dma_start(q_sb[:], Q_d[b])

            # views with the interleaved Lc mapping  i = p*8 + t
            c_tiles = c_sb[:].rearrange("d (p t) -> d t p", t=NT)  # [D, t, p]
            wv = w_sb[:].rearrange("p (t c) -> p t c", c=3 * D)
            w1 = wv[:, :, 0:D]
            w2 = wv[:, :, D:2 * D]
            w3 = wv[:, :, 2 * D:3 * D]

            # ---- Ct tiles: PE transpose groups of 4; keep PSUM f32 copy
            #      for U/v, write bf16 (+ones col) SBUF copy for Tu ----
            ct_sb = sb.tile([TL, NT * (D + 1)], ct_dt, tag="ct", name=f"ct{b}")
            ctv = ct_sb[:].rearrange("p (t c) -> p t c", c=D + 1)
            nc.vector.memset(ctv[:, :, D:D + 1], 1.0)
            w3ct = sb.tile([TL, NT * D], f32, tag="w3ct", name=f"w3ct{b}")
            w3ctv = w3ct[:].rearrange("p (t c) -> p t c", c=D)
            vtmp = sb.tile([TL, NT * D], f32, tag="vtmp", name=f"vtmp{b}")
            vtmpv = vtmp[:].rearrange("p (t c) -> p t c", c=D)
            for g in range(2):
                ctp = tp_ps.tile([TL, 4 * D], f32, tag="tp", name=f"ctp{b}_{g}")
                for k in range(4):
                    t = 4 * g + k
                    nc.tensor.transpose(
                        ctp[:, k * D:(k + 1) * D], c_tiles[:, t, :], ident[:])
                ctpv = ctp[:].rearrange("p (k c) -> p k c", c=D)
                gs = slice(4 * g, 4 * g + 4)
                nc.scalar.activation(ctv[:, gs, 0:D], ctpv, AF.Copy)
                nc.vector.tensor_tensor(out=w3ctv[:, gs, :], in0=w3[:, gs, :],
                                        in1=ctpv, op=ALU.mult)
                if not v_bf16:
                    nc.vector.tensor_tensor(out=vtmpv[:, gs, :],
                                            in0=w2[:, gs, :], in1=ctpv,
                                            op=ALU.mult)
            if v_bf16:
                # gpsimd is idle; feed it the v multiply from the bf16 ct
                nc.gpsimd.tensor_tensor(out=vtmpv, in0=w2,
                                        in1=ctv[:, :, 0:D], op=ALU.mult)
            v_all = small.tile([TL, NT], f32, tag="v", name=f"v{b}")
            nc.vector.tensor_reduce(v_all[:], vtmpv, axis=AX.X, op=ALU.add)
            expv = small.tile([TL, NT], f32, tag="expv", name=f"expv{b}")
            nc.scalar.activation(expv[:], v_all[:], AF.Exp)

            # ---- U = w3ct + w1, then U^T via PE transposes ----
            u_all = sb.tile([TL, NT * D], f32, tag="u", name=f"u{b}")
            u_allv = u_all[:].rearrange("p (t c) -> p t c", c=D)
            nc.vector.tensor_tensor(out=u_allv, in0=w3ctv, in1=w1, op=ALU.add)
            ut_sb = sb.tile([D, LC], f32r, tag="ut", name=f"ut{b}")
            for g in range(2):
                utp = tp_ps.tile([D, 4 * TL], f32, tag="tp", name=f"utp{b}_{g}")
                for k in range(4):
                    t = 4 * g + k
                    nc.tensor.transpose(
                        utp[:, k * TL:(k + 1) * TL],
                        u_all[:, t * D:(t + 1) * D],
                        ident[0:TL, 0:TL])
                nc.scalar.activation(ut_sb[:, g * 500:(g + 1) * 500],
                                     utp[:], AF.Copy)

            # ---- S0T [Lq, Lc] (permuted cols), f32r matmuls ----
            q_r = small.tile([D, LQ], f32r, tag="q_r", name=f"q_r{b}")
            nc.scalar.activation(q_r[:], q_sb[:], AF.Copy)
            s0t = mm_ps.tile([LQ, LC], f32, tag="mm", name=f"s0t{b}")
            nc.tensor.matmul(s0t[:, 0:512], q_r[:], ut_sb[:, 0:512],
                             start=True, stop=True)
            nc.tensor.matmul(s0t[:, 512:1000], q_r[:], ut_sb[:, 512:1000],
                             start=True, stop=True)

            # ---- E1T = exp(S0T) as f32r ----
            e1t_sb = sb.tile([LQ, LC], f32r, tag="e1t", name=f"e1t{b}")
            nc.scalar.activation(e1t_sb[:], s0t[:], AF.Exp)

            # ---- s1 = colsum(E1T) via ones-matmul; 1/s1; broadcast ----
            s1s0 = tp_ps.tile([1, 512], f32, tag="tp", name=f"s1s0_{b}")
            s1s1 = tp_ps.tile([1, 512], f32, tag="tp", name=f"s1s1_{b}")
            nc.tensor.matmul(s1s0[:, 0:512], ones_col[0:LQ, :],
                             e1t_sb[:, 0:512], start=True, stop=True)
            nc.tensor.matmul(s1s1[:, 0:488], ones_col[0:LQ, :],
                             e1t_sb[:, 512:1000], start=True, stop=True)
            s1rr = small.tile([1, LC], f32r, tag="s1rr", name=f"s1rr{b}")
            with nc.allow_low_precision(reason="f32r rounding is deliberate"):
                nc.vector.reciprocal(s1rr[:, 0:512], s1s0[:, 0:512])
                nc.vector.reciprocal(s1rr[:, 512:1000], s1s1[:, 0:488])
            s1bc = mm_ps.tile([LQ, LC], f32, tag="mm", name=f"s1bc{b}")
            nc.tensor.matmul(s1bc[:, 0:512], ones_row[:, 0:LQ],
                             s1rr[:, 0:512], start=True, stop=True)
            nc.tensor.matmul(s1bc[:, 512:1000], ones_row[:, 0:LQ],
                             s1rr[:, 512:1000], start=True, stop=True)
            s1t = sb.tile([LQ, LC], f32r, tag="s1t", name=f"s1t{b}")
            nc.vector.tensor_tensor(out=s1t[:], in0=e1t_sb[:], in1=s1bc[:],
                                    op=ALU.mult)

            # ---- E2 tiles = transpose(E1T) * exp(v), cast for Tu ----
            e2_all = sb.tile([TL, NT * LQ], e2_dt, tag="e2", name=f"e2{b}")
            e2v = e2_all[:].rearrange("p (t c) -> p t c", c=LQ)
            for g in range(2):
                e1p = tp_ps.tile([TL, 4 * LQ], f32r, tag="tp",
                                 name=f"e1p{b}_{g}")
                for k in range(4):
                    t = 4 * g + k
                    nc.tensor.transpose(
                        e1p[:, k * LQ:(k + 1) * LQ],
                        e1t_sb[:, t * TL:(t + 1) * TL],
                        identr[0:LQ, 0:LQ])
                scl = expv[:, 4 * g:4 * g + 4].unsqueeze(-1).to_broadcast(
                    (TL, 4, LQ))
                nc.vector.tensor_tensor(
                    out=e2v[:, 4 * g:4 * g + 4, :],
                    in0=e1p[:].rearrange("p (k c) -> p k c", c=LQ),
                    in1=scl, op=ALU.mult)

            # ---- Qt ----
            qtp = tp_ps.tile([LQ, D], f32, tag="tp", name=f"qtp{b}")
            nc.tensor.transpose(qtp[:], q_sb[:], ident[:])
            qt_sb = small.tile([LQ, D], f32r, tag="qt", name=f"qt{b}")
            nc.scalar.activation(qt_sb[:], qtp[:], AF.Copy)

            # ---- Tu = E2^T @ [Ct | 1]  (accumulate over tiles) ----
            tu = tp_ps.tile([LQ, D + 1], f32, tag="tp", name=f"tu{b}")
            for t in range(NT):
                nc.tensor.matmul(tu[:], e2v[:, t, :], ctv[:, t, :],
                                 start=(t == 0), stop=(t == NT - 1))
            s2r = small.tile([LQ, 1], f32, tag="s2r", name=f"s2r{b}")
            nc.vector.reciprocal(s2r[:], tu[:, D:D + 1])
            that_sb = small.tile([LQ, D], f32r, tag="that", name=f"that{b}")
            nc.vector.tensor_scalar_mul(that_sb[:], tu[:, 0:D], s2r[:])

            # ---- A^T and Bm^T ----
            at = mm_ps.tile([D, LC], f32, tag="mm", name=f"at{b}")
            nc.tensor.matmul(at[:, 0:512], qt_sb[:], s1t[:, 0:512],
                             start=True, stop=True)
            nc.tensor.matmul(at[:, 512:1000], qt_sb[:], s1t[:, 512:1000],
                             start=True, stop=True)
            bmt = mm_ps.tile([D, LC], f32, tag="mm", name=f"bmt{b}")
            nc.tensor.matmul(bmt[:, 0:512], that_sb[:], s1t[:, 0:512],
                             start=True, stop=True)
            nc.tensor.matmul(bmt[:, 512:1000], that_sb[:], s1t[:, 512:1000],
                             start=True, stop=True)

            # ---- outputs (unpermute Lc: psum col t*125+p -> true i=p*8+t) --
            atp = at[:].rearrange("d (t p) -> d p t", t=NT)
            bmtp = bmt[:].rearrange("d (t p) -> d p t", t=NT)
            c_out = c_sb[:].rearrange("d (p t) -> d p t", t=NT)
            nc.sync.dma_start(O_d[b, 0:D], c_sb[:])
            oa = outp.tile([D, LC], f32, tag="oa", name=f"oa{b}")
            nc.scalar.activation(oa[:].rearrange("d (p t) -> d p t", t=NT),
                                 atp, AF.Copy)
            nc.sync.dma_start(O_d[b, D:2 * D], oa[:])
            oca = outp.tile([D, LC], f32, tag="oca", name=f"oca{b}")
            nc.vector.tensor_tensor(
                out=oca[:].rearrange("d (p t) -> d p t", t=NT),
                in0=c_out, in1=atp, op=ALU.mult)
            nc.sync.dma_start(O_d[b, 2 * D:3 * D], oca[:])
            ocb = outp.tile([D, LC], f32, tag="ocb", name=f"ocb{b}")
            nc.vector.tensor_tensor(
                out=ocb[:].rearrange("d (p t) -> d p t", t=NT),
                in0=c_out, in1=bmtp, op=ALU.mult)
            nc.sync.dma_start(O_d[b, 3 * D:4 * D], ocb[:])

    nc.compile()
    return nc


def _get_nc(**kw):
    key = tuple(sorted(kw.items()))
    if key not in _cache:
        _cache[key] = _build(**kw)
    return _cache[key]


def kernel(C, Q, W, **build_kw):
    from concourse import bass_utils

    C = np.ascontiguousarray(C, np.float32)
    Q = np.ascontiguousarray(Q, np.float32)
    Wr = np.ascontiguousarray(W, np.float32).reshape(NCORES, NB, LC, 3 * D)
    Cs = C.reshape(NCORES, NB, D, LC)
    Qs = Q.reshape(NCORES, NB, D, LQ)

    nc = _get_nc(**build_kw)
    in_maps = [{"C": Cs[i], "Q": Qs[i], "W": Wr[i]} for i in range(NCORES)]
    res = bass_utils.run_bass_kernel_spmd(nc, in_maps,
                                          core_ids=list(range(NCORES)))
    out = np.concatenate([res.results[i]["OUT"] for i in range(NCORES)], 0)
    return out.astype(np.float32)
